# revision 1
# baseline (speedup 1.0000x reference)
"""Trainium2 Bass kernel for CustomBSplineLayer.

Computes out[b,o] = sum_{i,g} spline(x)[b,i,g] * coef[o,i,g] where
spline is an order-3 (cubic) B-spline basis on uniform knots applied to
tanh(x).

Math used here (validated against the reference recursion):
  u = 3.5*tanh(x) + 3.5           in (0, 7)
  basis_g(u) = M4(u - g)          cardinal cubic B-spline, g = 0..7
  M4(s) = (relu(2-|s-2|)^3 - 4*relu(1-|s-2|)^3) / 6
Plane g=7 is identically zero because its support starts at u=7 ==
tanh upper bound, so only 7 of 8 planes contribute (K = 7*1024 = 7168).

Per-core layout (data-parallel over batch, 8 cores x 512 rows):
  - host pre-transposes x so that tiles arrive as [i partitions, b cols]
  - basis planes computed in [i, b] layout feed the PE directly as the
    stationary (lhsT) operand; coef (host-rearranged to [g, i, o]) is the
    moving operand; out accumulates in PSUM as [b, o] across all 56
    k-tiles, then is copied out once.
  - matmul runs in float32r (tf32) which streams at 1 col/cycle for
    free-dim >= 256 (fp32 is 4 cycles/col).
"""

import sys

sys.path.insert(0, "/opt/trn_rl_repo")

import numpy as np
from contextlib import ExitStack

import concourse.bass as bass
import concourse.tile as tile
from concourse import bacc, mybir
from concourse.bass_utils import run_bass_kernel_spmd

F32 = mybir.dt.float32
F32R = mybir.dt.float32r
I32 = mybir.dt.int32
AF = mybir.ActivationFunctionType
OP = mybir.AluOpType

B, I, O = 4096, 1024, 1024
G = 7                    # active basis planes (plane 7 == 0)
NCORES = 8
BC = B // NCORES         # 512 batch rows per core
IT = I // 128            # 8 i-tiles
KT = IT * G              # 56 k-tiles of 128
WID = G * BC             # 3584: wide free-dim (7 planes x 512 b)

C6 = float(6.0 ** (-1.0 / 3.0))          # folds the 1/6 into p
C46 = float((4.0 / 6.0) ** (1.0 / 3.0))  # folds the 4/6 into q
KQ = float(C46 / C6)                     # q = relu(KQ*p - C46)

# mm dtype: F32R (tf32, fast) or F32 (exact, 4x slower PE)
MM_DT = F32R

LAST_RESULT = None  # BassKernelResults of the most recent run (for test.py)

_cache = {}


def _tf32_round(a: np.ndarray) -> np.ndarray:
    """Round fp32 to tf32 (10-bit mantissa), round-to-nearest-even."""
    bits = np.ascontiguousarray(a, dtype=np.float32).view(np.uint32).copy()
    lsb = (bits >> np.uint32(13)) & np.uint32(1)
    bits += np.uint32(0xFFF) + lsb
    bits &= np.uint32(0xFFFFE000)
    return bits.view(np.float32)


def _build_nc(repeats: int = 1):
    nc = bacc.Bacc("TRN2", target_bir_lowering=False, debug=False)
    xT = nc.dram_tensor("xT", [I, BC], F32, kind="ExternalInput").ap()
    coefT = nc.dram_tensor("coefT", [G, I, O], MM_DT, kind="ExternalInput").ap()
    y = nc.dram_tensor("y", [BC, O], F32, kind="ExternalOutput").ap()

    with tile.TileContext(nc) as tc, ExitStack() as ctx:
        xt_pool = ctx.enter_context(tc.tile_pool(name="xt", bufs=2))
        small = ctx.enter_context(tc.tile_pool(name="small", bufs=2))
        wide = ctx.enter_context(tc.tile_pool(name="wide", bufs=2))
        spl_pool = ctx.enter_context(tc.tile_pool(name="spl", bufs=2))
        rhs_pool = ctx.enter_context(tc.tile_pool(name="rhs", bufs=3))
        out_pool = ctx.enter_context(tc.tile_pool(name="ot", bufs=2))
        psum_pool = ctx.enter_context(
            tc.tile_pool(name="psum", bufs=1, space=bass.MemorySpace.PSUM)
        )

        consts = ctx.enter_context(tc.tile_pool(name="consts", bufs=1))
        bias_p = consts.tile([128, 1], F32, tag="bias_p", name="bias_p")
        nc.gpsimd.memset(bias_p[:], 2.0 * C6)
        bias_q = consts.tile([128, 1], F32, tag="bias_q", name="bias_q")
        nc.gpsimd.memset(bias_q[:], -C46)

        # 8 PSUM banks: [m-tile 0..3] x [o-half 0..1], each [128, 512] f32
        psum = [
            [
                psum_pool.tile(
                    [128, 512], F32, tag=f"ps{m}_{h}", name=f"ps{m}_{h}"
                )
                for h in range(2)
            ]
            for m in range(4)
        ]

        def emit_front(rep, it):
            """DMA + tanh + a-planes for i-tile `it` (stage A: ACT+DVE)."""
            xt = xt_pool.tile([128, BC], F32, tag="xt", name=f"xt{rep}_{it}")
            nc.sync.dma_start(xt[:], xT[it * 128 : (it + 1) * 128, :])
            t = small.tile([128, BC], F32, tag="t", name=f"t{rep}_{it}")
            nc.scalar.activation(t[:], xt[:], AF.Tanh)
            # w_g = u-(g+2) = 3.5*t + (1.5-g); one wide sign-bit clear (int
            # AND) turns all 7 planes into a_g = |w_g| at once
            aw = wide.tile([128, WID], F32, tag="a", name=f"aw{rep}_{it}")
            for g in range(G):
                nc.vector.tensor_scalar(
                    aw[:, g * BC : (g + 1) * BC],
                    t[:],
                    3.5,
                    float(1.5 - g),
                    OP.mult,
                    OP.add,
                )
            awi = aw[:].bitcast(I32)
            nc.vector.tensor_scalar(awi, awi, 0x7FFFFFFF, None, OP.bitwise_and)
            return aw

        def emit_mids(rep, it, aw, chunks=1):
            """ACT middle stage: p, q, p^2, q^2 for i-tile `it`."""
            pw = wide.tile([128, WID], F32, tag="p", name=f"pw{rep}_{it}")
            qw = wide.tile([128, WID], F32, tag="q", name=f"qw{rep}_{it}")
            p2 = wide.tile([128, WID], F32, tag="p2", name=f"p2{rep}_{it}")
            q2 = wide.tile([128, WID], F32, tag="q2", name=f"q2{rep}_{it}")
            cw = WID // chunks
            for c in range(chunks):
                s = slice(c * cw, (c + 1) * cw)
                nc.scalar.activation(
                    pw[:, s], aw[:, s], AF.Relu, bias=bias_p[:], scale=-C6
                )
                nc.scalar.activation(
                    qw[:, s], pw[:, s], AF.Relu, bias=bias_q[:], scale=KQ
                )
                nc.scalar.activation(p2[:, s], pw[:, s], AF.Square)
                nc.scalar.activation(q2[:, s], qw[:, s], AF.Square)
            return pw, qw, p2, q2

        def emit_cubes(rep, it, mids, chunks=1):
            """DVE cube stage: p2 *= p, q2 *= q (in place), spl = p3 - q3."""
            pw, qw, p2, q2 = mids
            spl = spl_pool.tile([128, WID], MM_DT, tag="spl", name=f"spl{rep}_{it}")
            cw = WID // chunks
            for c in range(chunks):
                s = slice(c * cw, (c + 1) * cw)
                nc.vector.tensor_tensor(p2[:, s], p2[:, s], pw[:, s], OP.mult)
                nc.vector.tensor_tensor(q2[:, s], q2[:, s], qw[:, s], OP.mult)
                # subtract writes an fp32r-typed tile: the DVE rounds to
                # tf32 on write, as the fp32r matmul requires of producers
                nc.vector.tensor_tensor(spl[:, s], p2[:, s], q2[:, s], OP.subtract)
            return spl

        def emit_matmuls(rep, it, spl, kt):
            for g in range(G):
                rhs = rhs_pool.tile(
                    [128, O], MM_DT, tag="rhs", name=f"rhs{rep}_{it}_{g}"
                )
                nc.sync.dma_start(rhs[:], coefT[g, it * 128 : (it + 1) * 128, :])
                first = kt == 0
                last = kt == KT - 1
                for m in range(4):
                    lhsT = spl[:, g * BC + m * 128 : g * BC + (m + 1) * 128]
                    for h in range(2):
                        nc.tensor.matmul(
                            psum[m][h][:],
                            lhsT,
                            rhs[:, h * 512 : (h + 1) * 512],
                            start=first,
                            stop=last,
                        )
                kt += 1
            return kt

        for _rep in range(repeats):
            # software-pipelined emission: within each block the DVE first
            # produces a(it), then (while ACT runs mids(it)) finishes the
            # cubes of it-1, whose matmuls follow immediately.
            kt = 0
            # i-tile 0 runs per-plane (chunks=G) so its first matmuls can
            # start ~20us earlier (deps are tracked per slice); later tiles
            # use full-wide ops.
            ch0 = G if _rep == 0 else 1
            aw = emit_front(_rep, 0)
            mids = emit_mids(_rep, 0, aw, chunks=ch0)
            for it in range(1, IT):
                aw = emit_front(_rep, it)
                prev_mids = mids
                mids = emit_mids(_rep, it, aw)
                spl = emit_cubes(_rep, it - 1, prev_mids, chunks=ch0 if it == 1 else 1)
                kt = emit_matmuls(_rep, it - 1, spl, kt)
            spl = emit_cubes(_rep, IT - 1, mids)
            kt = emit_matmuls(_rep, IT - 1, spl, kt)

            for m in range(4):
                ot = out_pool.tile([128, O], F32, tag="ot", name=f"ot{_rep}_{m}")
                for h in range(2):
                    nc.scalar.copy(ot[:, h * 512 : (h + 1) * 512], psum[m][h][:])
                nc.sync.dma_start(y[m * 128 : (m + 1) * 128, :], ot[:])

    nc.compile()
    return nc


def kernel(x: np.ndarray, coef: np.ndarray) -> np.ndarray:
    global LAST_RESULT
    x = np.asarray(x, dtype=np.float32)
    coef = np.asarray(coef, dtype=np.float32)
    assert x.shape == (B, I) and coef.shape == (O, I, 8)

    if "nc" not in _cache:
        _cache["nc"] = _build_nc()
    nc = _cache["nc"]

    xT = np.ascontiguousarray(x.T)  # [I, B]
    coefT = np.ascontiguousarray(coef.transpose(2, 1, 0)[:G])  # [7, I, O]
    if MM_DT == F32R:
        coefT = _tf32_round(coefT)
    in_maps = [
        {
            "xT": np.ascontiguousarray(xT[:, c * BC : (c + 1) * BC]),
            "coefT": coefT,
        }
        for c in range(NCORES)
    ]
    res = run_bass_kernel_spmd(nc, in_maps, list(range(NCORES)))
    LAST_RESULT = res
    out = np.concatenate([res.results[c]["y"] for c in range(NCORES)], axis=0)
    return np.ascontiguousarray(out.astype(np.float32))


if __name__ == "__main__":
    rng = np.random.default_rng(0)
    x = rng.standard_normal((B, I), dtype=np.float32)
    coef = rng.standard_normal((O, I, 8), dtype=np.float32) * 0.1
    out = kernel(x, coef)
    print("out", out.shape, out.dtype, float(np.abs(out).max()))



# revision 5
# speedup vs baseline: 1.4468x; 1.4468x over previous
"""Trainium2 Bass kernel for CustomBSplineLayer.

Computes out[b,o] = sum_{i,g} spline(x)[b,i,g] * coef[o,i,g] where
spline is an order-3 (cubic) B-spline basis on uniform knots applied to
tanh(x).

Math (validated against the reference recursion):
  u = 3.5*tanh(x) + 3.5              in (0, 7)
  basis_g(u) = M4(u - g)             cardinal cubic B-spline, g = 0..7
  M4(s) = (relu(2-|s-2|)^3 - 4*relu(1-|s-2|)^3) / 6
Plane g=7 is identically zero on (0,7), so only 7 of 8 planes
contribute (K = 7*1024 = 7168 per batch row).

With p = relu(2*C6 - C6*|s-2|) and q = relu(KQ*p - C46) (C6^3 = 1/6,
C46^3 = 4/6, KQ = C46/C6), M4 = p^3 - q^3 exactly.

Per-core layout (data-parallel over batch, 8 cores x 512 rows):
  - host pre-transposes x so tiles arrive as [i partitions, b cols];
    basis planes in [i, b] layout feed the PE directly as the stationary
    (lhsT) operand; coef (host-rearranged to [g, i, o], bf16) is the
    moving operand; out accumulates in PSUM as [b, o] across 56 k-tiles.
  - the whole per-plane basis evaluation runs as TWO single-pass custom
    DVE ops (8-slice fused ALU programs):
      opA: p = relu(2*C6 - |t*(3.5*C6) + C6*(1.5-g)|)   from t = tanh(x)
      opB: s = p^3 - relu(p*KQ - C46)^3                 written as bf16
    so the vector engine does 2 ops/plane instead of ~5 and the scalar
    engine only computes tanh.
  - both matmul operands are bf16 (same 1 col/cycle PE rate as tf32, but
    half the coef DMA traffic and FWL weight loads).
"""

import sys

sys.path.insert(0, "/opt/trn_rl_repo")

import numpy as np
import ml_dtypes
from contextlib import ExitStack

import concourse.bass as bass
import concourse.tile as tile
from concourse import bacc, mybir
from concourse.bass_utils import run_bass_kernel_spmd
import concourse.dve_ops as dve_ops
from concourse.dve_spec import (
    Spec,
    Src0,
    C0,
    C1,
    C2,
    Zero,
    relu,
    maxx,
    sq,
    lower,
    _has_src1,
)
from concourse.dve_uop import DveOpSpec

F32 = mybir.dt.float32
F32R = mybir.dt.float32r
BF16 = mybir.dt.bfloat16
AF = mybir.ActivationFunctionType

B, I, O = 4096, 1024, 1024
G = 7                    # active basis planes (plane 7 == 0)
NCORES = 8
BC = B // NCORES         # 512 batch rows per core
IT = I // 128            # 8 i-tiles
KT = IT * G              # 56 k-tiles of 128

C6 = float(6.0 ** (-1.0 / 3.0))          # folds the 1/6 into p
C46 = float((4.0 / 6.0) ** (1.0 / 3.0))  # folds the 4/6 into q
KQ = float(C46 / C6)                     # q = relu(KQ*p - C46)

# matmul dtype for both operands (PE streams 1 col/cycle for bf16, same
# as tf32, but coef DMA halves and bf16 weights get fast weight load)
MM_DT = BF16

LAST_RESULT = None  # BassKernelResults of the most recent run (for test.py)

_cache = {}


def _tf32_round(a: np.ndarray) -> np.ndarray:
    """Round fp32 to tf32 (10-bit mantissa), round-to-nearest-even."""
    bits = np.ascontiguousarray(a, dtype=np.float32).view(np.uint32).copy()
    lsb = (bits >> np.uint32(13)) & np.uint32(1)
    bits += np.uint32(0xFFF) + lsb
    bits &= np.uint32(0xFFFFE000)
    return bits.view(np.float32)


def _register_op(name: str, spec: Spec) -> "dve_ops.DveOp":
    """Register a custom DVE op at runtime (concourse keys the per-NEFF
    uop table and CoreSim reference off these module-level registries)."""
    for op in dve_ops.OPS:
        if op.name == name:
            return op
    row = dve_ops._CUSTOM_DVE_ROW_BASE + len(dve_ops.OPS)
    assert row < 0x20, "custom-DVE opcode rows exhausted"
    shas = {}
    for ver in ("v3", "v4"):
        try:
            uops = lower(spec, ver=ver)
            shas[ver] = DveOpSpec(
                name=name, opcode=row, uops=uops, rd1_en=_has_src1(spec)
            ).sha(ver)
        except Exception:
            pass
    op = dve_ops.DveOp(name, spec, subdim=False, uops_sha=shas)
    dve_ops.OPS.append(op)
    dve_ops.CUSTOM_DVE_SPECS[name] = spec
    dve_ops._SUB_OPCODE_FOR_NAME[name] = row
    return op


# opA: p = relu(imm2 - |in0*s0 + s1|)
_w = Src0 * C0 + C1
OPA = _register_op(
    "BSPLINE_P_ANT",
    Spec(
        body=relu(C2 - maxx(_w, Zero - _w)),
        reference=lambda in0, in1, s0, s1, imm2: np.maximum(
            imm2 - np.abs(in0.astype(np.float32) * s0 + s1), 0.0
        ).astype(np.float32),
    ),
)

# opB: s = in0^3 - relu(in0*s0 - s1)^3
_q = relu(Src0 * C0 - C1)
OPB = _register_op(
    "BSPLINE_CUBE_ANT",
    Spec(
        body=sq(Src0) * Src0 - sq(_q) * _q,
        reference=lambda in0, in1, s0, s1, imm2: (
            in0.astype(np.float32) ** 3
            - np.maximum(in0.astype(np.float32) * s0 - s1, 0.0) ** 3
        ).astype(np.float32),
    ),
)


def _build_nc(repeats: int = 1):
    nc = bacc.Bacc("TRN2", target_bir_lowering=False, debug=False)
    xT = nc.dram_tensor("xT", [I, BC], F32, kind="ExternalInput").ap()
    coefT = nc.dram_tensor("coefT", [G, I, O], MM_DT, kind="ExternalInput").ap()
    y = nc.dram_tensor("y", [BC, O], F32, kind="ExternalOutput").ap()

    with tile.TileContext(nc) as tc, ExitStack() as ctx:
        xt_pool = ctx.enter_context(tc.tile_pool(name="xt", bufs=2))
        t_pool = ctx.enter_context(tc.tile_pool(name="t", bufs=2))
        p_pool = ctx.enter_context(tc.tile_pool(name="p", bufs=2))
        pc_pool = ctx.enter_context(tc.tile_pool(name="pc", bufs=2))
        spl_pool = ctx.enter_context(tc.tile_pool(name="spl", bufs=4))
        rhs_pool = ctx.enter_context(tc.tile_pool(name="rhs", bufs=4))
        out_pool = ctx.enter_context(tc.tile_pool(name="ot", bufs=2))
        psum_pool = ctx.enter_context(
            tc.tile_pool(name="psum", bufs=1, space=bass.MemorySpace.PSUM)
        )

        # 8 PSUM banks: [m-tile 0..3] x [o-half 0..1], each [128, 512] f32
        psum = [
            [
                psum_pool.tile([128, 512], F32, tag=f"ps{m}_{h}", name=f"ps{m}_{h}")
                for h in range(2)
            ]
            for m in range(4)
        ]

        def emit_plane(rep, it, g, t, rhs, kt):
            """Full-width basis plane + its 8 matmuls."""
            p = p_pool.tile([128, BC], F32, tag="p", name=f"p{rep}_{it}_{g}")
            nc.vector._custom_dve(
                OPA, out=p[:], in0=t[:],
                s0=3.5 * C6, s1=C6 * (1.5 - g), imm2=2.0 * C6,
            )
            spl = spl_pool.tile([128, BC], MM_DT, tag="spl", name=f"spl{rep}_{it}_{g}")
            nc.vector._custom_dve(OPB, out=spl[:], in0=p[:], s0=KQ, s1=C46)
            first = kt == 0
            last = kt == KT - 1
            for m in range(4):
                lhsT = spl[:, m * 128 : (m + 1) * 128]
                for h in range(2):
                    nc.tensor.matmul(
                        psum[m][h][:],
                        lhsT,
                        rhs[:, h * 512 : (h + 1) * 512],
                        start=first,
                        stop=last,
                    )

        def emit_first_tile_chunked(rep):
            """i-tile 0 in 128-col chunks, plane-major: the first matmul can
            issue ~1.5us into the kernel instead of waiting for full-width
            tanh+basis, and the PE ramps while coef DMA streams."""
            xt = xt_pool.tile([128, BC], F32, tag="xt", name=f"xt{rep}_0")
            t = t_pool.tile([128, BC], F32, tag="t", name=f"t{rep}_0")
            for c in range(4):
                sl = slice(c * 128, (c + 1) * 128)
                nc.sync.dma_start(xt[:, sl], xT[0:128, c * 128 : (c + 1) * 128])
                nc.scalar.activation(t[:, sl], xt[:, sl], AF.Tanh)
            rhs_g = {}
            for g in range(G):
                r = rhs_pool.tile([128, O], MM_DT, tag="rhs", name=f"rhs{rep}_0_{g}")
                nc.sync.dma_start(r[:], coefT[g, 0:128, :])
                rhs_g[g] = r
                spl = spl_pool.tile([128, BC], MM_DT, tag="spl", name=f"spl{rep}_0_{g}")
                for c in range(4):
                    sl = slice(c * 128, (c + 1) * 128)
                    p = pc_pool.tile([128, 128], F32, tag="pc", name=f"pc{rep}_{g}_{c}")
                    nc.vector._custom_dve(
                        OPA, out=p[:], in0=t[:, sl],
                        s0=3.5 * C6, s1=C6 * (1.5 - g), imm2=2.0 * C6,
                    )
                    nc.vector._custom_dve(OPB, out=spl[:, sl], in0=p[:], s0=KQ, s1=C46)
                    for h in range(2):
                        nc.tensor.matmul(
                            psum[c][h][:],
                            spl[:, sl],
                            rhs_g[g][:, h * 512 : (h + 1) * 512],
                            start=(g == 0),
                            stop=False,
                        )

        def emit_front(rep, it):
            """DMA + tanh for one i-tile; returns the t tile."""
            xt = xt_pool.tile([128, BC], F32, tag="xt", name=f"xt{rep}_{it}")
            nc.sync.dma_start(xt[:], xT[it * 128 : (it + 1) * 128, :])
            t = t_pool.tile([128, BC], F32, tag="t", name=f"t{rep}_{it}")
            nc.scalar.activation(t[:], xt[:], AF.Tanh)
            return t

        next_front = None  # pre-emitted tanh tile for the next rep's i-tile 0
        for _rep in range(repeats):
            kt = 0
            for it in range(IT - 1):
                if it == 0 and _rep == 0:
                    emit_first_tile_chunked(_rep)
                    kt += G
                    continue
                if it == 0:
                    t = next_front
                    next_front = None
                else:
                    t = emit_front(_rep, it)
                for g in range(G):
                    rhs = rhs_pool.tile(
                        [128, O], MM_DT, tag="rhs", name=f"rhs{_rep}_{it}_{g}"
                    )
                    nc.sync.dma_start(rhs[:], coefT[g, it * 128 : (it + 1) * 128, :])
                    emit_plane(_rep, it, g, t, rhs, kt)
                    kt += 1

            # Last i-tile: produce all 7 planes first, then run the matmuls
            # m-outer so bank m finishes 7*(3-m) matmul-pairs early and its
            # PSUM drain + y DMA overlap the remaining stream.  The next
            # rep's tanh is emitted before the drain copies so the ACT FIFO
            # doesn't head-of-line-block the next rep's basis pipeline.
            it = IT - 1
            t = emit_front(_rep, it)
            spl_g, rhs_g = {}, {}
            for g in range(G):
                rhs = rhs_pool.tile(
                    [128, O], MM_DT, tag="rhs", name=f"rhs{_rep}_{it}_{g}"
                )
                nc.sync.dma_start(rhs[:], coefT[g, it * 128 : (it + 1) * 128, :])
                rhs_g[g] = rhs
                p = p_pool.tile([128, BC], F32, tag="p", name=f"p{_rep}_{it}_{g}")
                nc.vector._custom_dve(
                    OPA, out=p[:], in0=t[:],
                    s0=3.5 * C6, s1=C6 * (1.5 - g), imm2=2.0 * C6,
                )
                spl = spl_pool.tile(
                    [128, BC], MM_DT, tag="spl", name=f"spl{_rep}_{it}_{g}"
                )
                nc.vector._custom_dve(OPB, out=spl[:], in0=p[:], s0=KQ, s1=C46)
                spl_g[g] = spl
            if _rep + 1 < repeats:
                next_front = emit_front(_rep + 1, 0)
            for m in range(4):
                for g in range(G):
                    lhsT = spl_g[g][:, m * 128 : (m + 1) * 128]
                    for h in range(2):
                        nc.tensor.matmul(
                            psum[m][h][:],
                            lhsT,
                            rhs_g[g][:, h * 512 : (h + 1) * 512],
                            start=False,
                            stop=(g == G - 1),
                        )
                ot = out_pool.tile([128, O], F32, tag="ot", name=f"ot{_rep}_{m}")
                for h in range(2):
                    nc.scalar.copy(ot[:, h * 512 : (h + 1) * 512], psum[m][h][:])
                nc.sync.dma_start(y[m * 128 : (m + 1) * 128, :], ot[:])

    nc.compile()
    return nc


def kernel(x: np.ndarray, coef: np.ndarray) -> np.ndarray:
    global LAST_RESULT
    x = np.asarray(x, dtype=np.float32)
    coef = np.asarray(coef, dtype=np.float32)
    assert x.shape == (B, I) and coef.shape == (O, I, 8)

    if "nc" not in _cache:
        _cache["nc"] = _build_nc()
    nc = _cache["nc"]

    xT = np.ascontiguousarray(x.T)  # [I, B]
    coefT = np.ascontiguousarray(
        coef.transpose(2, 1, 0)[:G].astype(ml_dtypes.bfloat16)
    )  # [7, I, O] bf16
    in_maps = [
        {
            "xT": np.ascontiguousarray(xT[:, c * BC : (c + 1) * BC]),
            "coefT": coefT,
        }
        for c in range(NCORES)
    ]
    res = run_bass_kernel_spmd(nc, in_maps, list(range(NCORES)))
    LAST_RESULT = res
    out = np.concatenate([res.results[c]["y"] for c in range(NCORES)], axis=0)
    return np.ascontiguousarray(out.astype(np.float32))


if __name__ == "__main__":
    rng = np.random.default_rng(0)
    x = rng.standard_normal((B, I), dtype=np.float32)
    coef = rng.standard_normal((O, I, 8), dtype=np.float32) * 0.1
    out = kernel(x, coef)
    print("out", out.shape, out.dtype, float(np.abs(out).max()))
